# revision 7
# baseline (speedup 1.0000x reference)
"""HalfKP input layer (dual GEMV + bias + relu) on 8 Trainium2 NeuronCores.

out[512] = concat(relu(W_my @ x[:41024] + b_my), relu(W_opp @ x[41024:] + b_opp))

Sharding: 512 output rows split 64 rows/core (output-feature parallel; cores
0-3 handle W_my, 4-7 handle W_opp).  Per core the [64, 41024] shard is
host-repacked into [128, 8*2564]: partition p = rr*16 + b holds row
(t*8 + rr)'s k-block b (kb=2564) at free offset t*2564.  The device streams W
contiguously (10.3KB runs per partition per DMA), runs 8 fused
multiply+reduce custom-DVE ops (TENSOR_TENSOR_REDUCE against a [128, 2564]
x-block tile, bias seeded via s0), contracts the 16 k-block partials per row
with one tiny PE matmul, applies relu on DVE, and writes a [8, 8] result per
core.  Memory-roofline bound: ~10.5 MB HBM reads per core.
"""

import numpy as np

K = 41024          # features per side
B = 16             # k-blocks per row
KB = K // B        # elements per k-block
R = 128 // B       # 4 rows processed per DVE op
T = 64 // R        # 16 DVE ops (row groups) per core
CHUNK = 1          # TTR ops per W DMA (10.3KB contiguous runs per partition)
XCOLS = KB + R + T  # xq | mask[4] | seed[16]
N_CORES = 8
ROWS_PER_CORE = 64

_compiled = None


def _build_nc():
    import concourse.bacc as bacc
    import concourse.mybir as mybir
    import concourse.tile as tile
    from concourse.dve_ops import TENSOR_TENSOR_REDUCE

    F32 = mybir.dt.float32

    nc = bacc.Bacc("TRN2", target_bir_lowering=False, debug=False)

    wt_d = nc.dram_tensor("wt", [128, T * KB], F32, kind="ExternalInput")
    xqp_d = nc.dram_tensor("xqp", [128, XCOLS], F32, kind="ExternalInput")
    out_d = nc.dram_tensor("out", [R, T], F32, kind="ExternalOutput")

    def ttr(w_ap, xq_ap, seed_ap, acc_ap, prod_ap):
        # out = in0*in1*s1; accum = s0 + sum(out)  (custom-DVE ucode op)
        nc.vector._custom_dve(
            TENSOR_TENSOR_REDUCE,
            out=prod_ap,
            in0=w_ap,
            in1=xq_ap,
            s0=seed_ap,
            s1=1.0,
            accum_out=acc_ap,
        )

    n_chunks = T // CHUNK
    with tile.TileContext(nc) as tc:
        with (
            tc.tile_pool(name="const", bufs=1) as constp,
            tc.tile_pool(name="w", bufs=n_chunks + 1) as wp,
            tc.tile_pool(name="scratch", bufs=1) as sp,
            tc.tile_pool(name="ps", bufs=1, space="PSUM") as psp,
        ):
            # xqp first: its completion gates the first TTR, and the sync
            # HWDGE ring drains FIFO, so it must not queue behind W chunks
            xqp = constp.tile([128, XCOLS], F32, tag="xqp")
            nc.sync.dma_start(xqp[:], xqp_d[:])
            xq = xqp[:, 0:KB]
            mask = xqp[:, KB : KB + R]
            seed = xqp[:, KB + R : KB + R + T]

            acc = constp.tile([128, T], F32, tag="acc")
            prod = sp.tile([128, KB], F32, tag="prod")

            for c in range(n_chunks):
                w_sb = wp.tile([128, CHUNK * KB], F32, tag="w")
                nc.sync.dma_start(
                    w_sb[:], wt_d[:, c * CHUNK * KB : (c + 1) * CHUNK * KB]
                )
                for j in range(CHUNK):
                    t = c * CHUNK + j
                    ttr(
                        w_sb[:, j * KB : (j + 1) * KB],
                        xq,
                        seed[:, t : t + 1],
                        acc[:, t : t + 1],
                        prod[:],
                    )

            ps = psp.tile([R, T], F32, tag="ps")
            nc.tensor.matmul(ps[:], lhsT=mask, rhs=acc[:], start=True, stop=True)
            out_sb = sp.tile([R, T], F32, tag="out")
            nc.vector.tensor_scalar_max(out_sb[:], ps[:], 0.0)
            nc.sync.dma_start(out_d[:], out_sb[:])

    nc.compile()
    return nc


def _get_nc():
    global _compiled
    if _compiled is None:
        _compiled = _build_nc()
    return _compiled


def make_in_maps(input, W_my, b_my, W_opp, b_opp):
    """Host-side sharding: per-core input dicts."""
    x = np.ascontiguousarray(input, dtype=np.float32)
    Wcat = np.concatenate(
        [np.asarray(W_my, np.float32), np.asarray(W_opp, np.float32)], axis=0
    )
    bcat = np.concatenate(
        [np.asarray(b_my, np.float32), np.asarray(b_opp, np.float32)]
    )

    mask = (np.arange(128)[:, None] // B == np.arange(R)[None, :]).astype(np.float32)

    in_maps = []
    for c in range(N_CORES):
        Wsh = Wcat[c * ROWS_PER_CORE : (c + 1) * ROWS_PER_CORE]  # [64, K]
        xs = x[:K] if c < 4 else x[K:]
        # wt[p = rr*B + b, t*KB + j] = Wsh[t*R + rr, b*KB + j]
        wt = np.ascontiguousarray(
            Wsh.reshape(T, R, B, KB).transpose(1, 2, 0, 3).reshape(128, T * KB)
        )
        bsh = bcat[c * ROWS_PER_CORE : (c + 1) * ROWS_PER_CORE]
        seed = np.zeros((128, T), np.float32)
        # partition rr*B (b == 0) seeds the bias for row t*R + rr
        seed[np.arange(R) * B, :] = bsh.reshape(T, R).T
        xqp = np.empty((128, XCOLS), np.float32)
        xqp[:, 0:KB] = np.tile(xs.reshape(B, KB), (R, 1))
        xqp[:, KB : KB + R] = mask
        xqp[:, KB + R :] = seed
        in_maps.append({"wt": wt, "xqp": xqp})
    return in_maps


def gather_output(results):
    """results: list of per-core dicts with 'out' [R, T] -> full [512]."""
    outs = []
    for c in range(N_CORES):
        o = np.asarray(results[c]["out"], np.float32)  # [R, T]
        outs.append(o.T.ravel())  # row r = t*R + rr
    return np.concatenate(outs)


def run_on_hw(in_maps, trace=False, **kwargs):
    from concourse.bass_utils import run_bass_kernel_spmd

    nc = _get_nc()
    return run_bass_kernel_spmd(
        nc, in_maps, core_ids=list(range(N_CORES)), trace=trace, **kwargs
    )


def kernel(input, W_my, b_my, W_opp, b_opp):
    in_maps = make_in_maps(input, W_my, b_my, W_opp, b_opp)
    res = run_on_hw(in_maps)
    return gather_output(res.results)


# revision 8
# speedup vs baseline: 1.0295x; 1.0295x over previous
"""HalfKP input layer (dual GEMV + bias + relu) on 8 Trainium2 NeuronCores.

out[512] = concat(relu(W_my @ x[:41024] + b_my), relu(W_opp @ x[41024:] + b_opp))

Sharding: 512 output rows split 64 rows/core (output-feature parallel; cores
0-3 handle W_my, 4-7 handle W_opp).  Per core the [64, 41024] shard is
host-repacked into [128, 8*2564]: partition p = rr*16 + b holds row
(t*8 + rr)'s k-block b (kb=2564) at free offset t*2564.  The device streams W
contiguously (10.3KB runs per partition per DMA), runs 8 fused
multiply+reduce custom-DVE ops (TENSOR_TENSOR_REDUCE against a [128, 2564]
x-block tile, bias seeded via s0), contracts the 16 k-block partials per row
with one tiny PE matmul, applies relu on DVE, and writes a [8, 8] result per
core.  Memory-roofline bound: ~10.5 MB HBM reads per core.
"""

import numpy as np

K = 41024          # features per side
B = 32             # k-blocks per row
KB = K // B        # 1282 elements per k-block
R = 128 // B       # 4 rows processed per DVE op
T = 64 // R        # 16 DVE ops (row groups) per core
CHUNK = 2          # TTR ops per W DMA (10.3KB contiguous runs per partition)
XCOLS = KB + R + T  # xq | mask[4] | seed[16]
N_CORES = 8
ROWS_PER_CORE = 64

_compiled = None


def _build_nc():
    import concourse.bacc as bacc
    import concourse.mybir as mybir
    import concourse.tile as tile
    from concourse.dve_ops import TENSOR_TENSOR_REDUCE

    F32 = mybir.dt.float32

    nc = bacc.Bacc("TRN2", target_bir_lowering=False, debug=False)

    wt_d = nc.dram_tensor("wt", [128, T * KB], F32, kind="ExternalInput")
    xqp_d = nc.dram_tensor("xqp", [128, XCOLS], F32, kind="ExternalInput")
    out_d = nc.dram_tensor("out", [R, T], F32, kind="ExternalOutput")

    def ttr(w_ap, xq_ap, seed_ap, acc_ap, prod_ap):
        # out = in0*in1*s1; accum = s0 + sum(out)  (custom-DVE ucode op)
        nc.vector._custom_dve(
            TENSOR_TENSOR_REDUCE,
            out=prod_ap,
            in0=w_ap,
            in1=xq_ap,
            s0=seed_ap,
            s1=1.0,
            accum_out=acc_ap,
        )

    n_chunks = T // CHUNK
    with tile.TileContext(nc) as tc:
        with (
            tc.tile_pool(name="const", bufs=1) as constp,
            tc.tile_pool(name="w", bufs=n_chunks + 1) as wp,
            tc.tile_pool(name="scratch", bufs=1) as sp,
            tc.tile_pool(name="ps", bufs=1, space="PSUM") as psp,
        ):
            # xqp rides the scalar (ACT) HWDGE ring so the W stream on the
            # sync ring starts immediately and both make progress in parallel
            xqp = constp.tile([128, XCOLS], F32, tag="xqp")
            nc.scalar.dma_start(xqp[:], xqp_d[:])
            xq = xqp[:, 0:KB]
            mask = xqp[:, KB : KB + R]
            seed = xqp[:, KB + R : KB + R + T]

            acc = constp.tile([128, T], F32, tag="acc")
            prod = sp.tile([128, KB], F32, tag="prod")

            for c in range(n_chunks):
                w_sb = wp.tile([128, CHUNK * KB], F32, tag="w")
                nc.sync.dma_start(
                    w_sb[:], wt_d[:, c * CHUNK * KB : (c + 1) * CHUNK * KB]
                )
                for j in range(CHUNK):
                    t = c * CHUNK + j
                    ttr(
                        w_sb[:, j * KB : (j + 1) * KB],
                        xq,
                        seed[:, t : t + 1],
                        acc[:, t : t + 1],
                        prod[:],
                    )

            ps = psp.tile([R, T], F32, tag="ps")
            nc.tensor.matmul(ps[:], lhsT=mask, rhs=acc[:], start=True, stop=True)
            out_sb = sp.tile([R, T], F32, tag="out")
            nc.vector.tensor_scalar_max(out_sb[:], ps[:], 0.0)
            nc.sync.dma_start(out_d[:], out_sb[:])

    nc.compile()
    return nc


def _get_nc():
    global _compiled
    if _compiled is None:
        _compiled = _build_nc()
    return _compiled


def make_in_maps(input, W_my, b_my, W_opp, b_opp):
    """Host-side sharding: per-core input dicts."""
    x = np.ascontiguousarray(input, dtype=np.float32)
    Wcat = np.concatenate(
        [np.asarray(W_my, np.float32), np.asarray(W_opp, np.float32)], axis=0
    )
    bcat = np.concatenate(
        [np.asarray(b_my, np.float32), np.asarray(b_opp, np.float32)]
    )

    mask = (np.arange(128)[:, None] // B == np.arange(R)[None, :]).astype(np.float32)

    in_maps = []
    for c in range(N_CORES):
        Wsh = Wcat[c * ROWS_PER_CORE : (c + 1) * ROWS_PER_CORE]  # [64, K]
        xs = x[:K] if c < 4 else x[K:]
        # wt[p = rr*B + b, t*KB + j] = Wsh[t*R + rr, b*KB + j]
        wt = np.ascontiguousarray(
            Wsh.reshape(T, R, B, KB).transpose(1, 2, 0, 3).reshape(128, T * KB)
        )
        bsh = bcat[c * ROWS_PER_CORE : (c + 1) * ROWS_PER_CORE]
        seed = np.zeros((128, T), np.float32)
        # partition rr*B (b == 0) seeds the bias for row t*R + rr
        seed[np.arange(R) * B, :] = bsh.reshape(T, R).T
        xqp = np.empty((128, XCOLS), np.float32)
        xqp[:, 0:KB] = np.tile(xs.reshape(B, KB), (R, 1))
        xqp[:, KB : KB + R] = mask
        xqp[:, KB + R :] = seed
        in_maps.append({"wt": wt, "xqp": xqp})
    return in_maps


def gather_output(results):
    """results: list of per-core dicts with 'out' [R, T] -> full [512]."""
    outs = []
    for c in range(N_CORES):
        o = np.asarray(results[c]["out"], np.float32)  # [R, T]
        outs.append(o.T.ravel())  # row r = t*R + rr
    return np.concatenate(outs)


def run_on_hw(in_maps, trace=False, **kwargs):
    from concourse.bass_utils import run_bass_kernel_spmd

    nc = _get_nc()
    return run_bass_kernel_spmd(
        nc, in_maps, core_ids=list(range(N_CORES)), trace=trace, **kwargs
    )


def kernel(input, W_my, b_my, W_opp, b_opp):
    in_maps = make_in_maps(input, W_my, b_my, W_opp, b_opp)
    res = run_on_hw(in_maps)
    return gather_output(res.results)
